# revision 7
# baseline (speedup 1.0000x reference)
"""Trainium2 Bass kernel for nn_AttentionBlock (GroupNorm -> QKV -> MHA -> proj -> residual).

Sharding: pure data-parallel over batch. B=16 across 8 cores -> 2 batches/core.
No collectives needed. Weights replicated, host pre-transposed/packed.

Self-contained: hardcodes shapes B=16, C=512, H=W=32 (T=1024), HEADS=8, GROUPS=32.
"""
import sys
import os

for _p in ("/opt/trn_rl_repo", "/root/.axon_site/_ro/trn_rl_repo"):
    if os.path.isdir(_p) and _p not in sys.path:
        sys.path.insert(0, _p)

import numpy as np
import ml_dtypes
from contextlib import ExitStack

import concourse.bass as bass
import concourse.tile as tile
from concourse import mybir, bacc
from concourse.bass_utils import run_bass_kernel_spmd

F32 = mybir.dt.float32
BF16 = mybir.dt.bfloat16

NCORES = 8
B = 16
C = 512
T = 1024
HEADS = 8
GROUPS = 32
CH = C // HEADS          # 64
NB = B // NCORES         # 2 batches per core
NCK = C // 128           # 4 channel chunks
EPS = 1e-5
SCALE2 = float(CH) ** -0.5  # (ch^-0.25)^2 folded into exp scale


def build_graph():
    nc = bacc.Bacc("TRN2", target_bir_lowering=False, debug=False,
                   num_devices=NCORES)

    x_d = nc.dram_tensor("x", [NB, C, T], F32, kind="ExternalInput").ap()
    wqkT_d = nc.dram_tensor("wqkT", [C, HEADS * 128], BF16, kind="ExternalInput").ap()
    wvT_d = nc.dram_tensor("wvT", [C, HEADS * 65], BF16, kind="ExternalInput").ap()
    wpT_d = nc.dram_tensor("wpT", [C, C], BF16, kind="ExternalInput").ap()
    bqk_d = nc.dram_tensor("bqk", [128, HEADS], F32, kind="ExternalInput").ap()
    bv_d = nc.dram_tensor("bv", [HEADS * 65], F32, kind="ExternalInput").ap()
    bp_d = nc.dram_tensor("bp", [128, NCK], F32, kind="ExternalInput").ap()
    gns_d = nc.dram_tensor("gns", [128, NCK], F32, kind="ExternalInput").ap()
    gnb_d = nc.dram_tensor("gnb", [128, NCK], F32, kind="ExternalInput").ap()
    gmask_d = nc.dram_tensor("gmask", [128, 8], F32, kind="ExternalInput").ap()
    gmaskT_d = nc.dram_tensor("gmaskT", [8, 128], F32, kind="ExternalInput").ap()
    out_d = nc.dram_tensor("out", [NB, C, T], F32, kind="ExternalOutput").ap()

    MULT = mybir.AluOpType.mult
    ADD = mybir.AluOpType.add
    SUB = mybir.AluOpType.subtract
    Exp = mybir.ActivationFunctionType.Exp
    Sqrt = mybir.ActivationFunctionType.Sqrt

    with tile.TileContext(nc) as tc, ExitStack() as ctx:
        consts = ctx.enter_context(tc.tile_pool(name="consts", bufs=1))
        xp = ctx.enter_context(tc.tile_pool(name="xp", bufs=2))
        xnp = ctx.enter_context(tc.tile_pool(name="xnp", bufs=2))
        qkp = ctx.enter_context(tc.tile_pool(name="qkp", bufs=4))
        vp = ctx.enter_context(tc.tile_pool(name="vp", bufs=2))
        pp = ctx.enter_context(tc.tile_pool(name="pp", bufs=2))
        ap_ = ctx.enter_context(tc.tile_pool(name="ap", bufs=2))
        op = ctx.enter_context(tc.tile_pool(name="op", bufs=4))
        stat = ctx.enter_context(tc.tile_pool(name="stat", bufs=2))
        rzp = ctx.enter_context(tc.tile_pool(name="rzp", bufs=2))
        psA = ctx.enter_context(tc.tile_pool(name="psA", bufs=2, space="PSUM"))
        psB = ctx.enter_context(tc.tile_pool(name="psB", bufs=2, space="PSUM"))

        # ---- constants (loaded once) ----
        wqkT_t = consts.tile([128, NCK, HEADS * 128], BF16)
        nc.sync.dma_start(wqkT_t, wqkT_d.rearrange("(o p) n -> p o n", p=128))
        wvT_t = consts.tile([128, NCK, HEADS * 65], BF16)
        nc.sync.dma_start(wvT_t, wvT_d.rearrange("(o p) n -> p o n", p=128))
        wpT_t = consts.tile([128, NCK, C], BF16)
        nc.sync.dma_start(wpT_t, wpT_d.rearrange("(o p) n -> p o n", p=128))
        bqk_t = consts.tile([128, HEADS], F32)
        nc.sync.dma_start(bqk_t, bqk_d)
        bv_t = consts.tile([128, HEADS * 65], F32)
        nc.sync.dma_start(bv_t, bv_d[None, :].to_broadcast([128, HEADS * 65]))
        bp_t = consts.tile([128, NCK], F32)
        nc.sync.dma_start(bp_t, bp_d)
        gns_t = consts.tile([128, NCK], F32)
        nc.sync.dma_start(gns_t, gns_d)
        gnb_t = consts.tile([128, NCK], F32)
        nc.sync.dma_start(gnb_t, gnb_d)
        gmask_t = consts.tile([128, 8], F32)
        nc.sync.dma_start(gmask_t, gmask_d)
        gmaskT_t = consts.tile([8, 128], F32)
        nc.sync.dma_start(gmaskT_t, gmaskT_d)
        eps_t = consts.tile([8, 1], F32)
        nc.vector.memset(eps_t, EPS)

        for b in range(NB):
            # ---- load x ----
            x_t = xp.tile([128, NCK, T], F32, tag="x")
            nc.sync.dma_start(x_t, x_d[b].rearrange("(o p) t -> p o t", p=128))

            # ---- GroupNorm stats ----
            mv = stat.tile([128, NCK, 2], F32, tag="mv")
            for ck in range(NCK):
                st = stat.tile([128, 2, 6], F32, tag="bnst")
                nc.vector.bn_stats(st[:, 0, :], x_t[:, ck, 0:512])
                nc.vector.bn_stats(st[:, 1, :], x_t[:, ck, 512:1024])
                nc.vector.bn_aggr(mv[:, ck, :], st)
            # me2: col0 = mean_c, col1 = E[x^2]_c = var_c + mean_c^2
            me2 = stat.tile([128, NCK, 2], F32, tag="me2")
            sq = stat.tile([128, NCK, 1], F32, tag="sq")
            nc.vector.tensor_copy(me2[:, :, 0:1], mv[:, :, 0:1])
            nc.vector.tensor_mul(sq, mv[:, :, 0:1], mv[:, :, 0:1])
            nc.vector.tensor_add(me2[:, :, 1:2], mv[:, :, 1:2], sq)

            # group aggregation: per chunk [8 groups, 2] = gmask.T @ me2 ; /16
            st8 = stat.tile([8, NCK, 2], F32, tag="st8")
            for ck in range(NCK):
                gps = psA.tile([8, 2], F32, tag="big")
                nc.tensor.matmul(gps, lhsT=gmask_t, rhs=me2[:, ck, :],
                                 start=True, stop=True)
                nc.vector.tensor_scalar_mul(st8[:, ck, :], gps, 1.0 / 16.0)
            # rstd_g = 1/sqrt(var_g + eps); var_g = E2_g - mean_g^2
            msq8 = stat.tile([8, NCK, 1], F32, tag="msq8")
            nc.vector.tensor_mul(msq8, st8[:, :, 0:1], st8[:, :, 0:1])
            var8 = stat.tile([8, NCK, 1], F32, tag="var8")
            nc.vector.tensor_sub(var8, st8[:, :, 1:2], msq8)
            sd8 = stat.tile([8, NCK, 1], F32, tag="sd8")
            nc.scalar.activation(sd8, var8, Sqrt, bias=eps_t)
            grp2 = stat.tile([8, NCK, 2], F32, tag="grp2")
            nc.vector.reciprocal(grp2[:, :, 1:2], sd8)
            nc.vector.tensor_copy(grp2[:, :, 0:1], st8[:, :, 0:1])

            # broadcast group stats back to channels; A = rstd*gamma, B = beta - mean*A
            ab = stat.tile([128, NCK, 2], F32, tag="ab")
            xn_t = xnp.tile([128, NCK, T], BF16, tag="xn")
            for ck in range(NCK):
                bcp = psA.tile([128, 2], F32, tag="big")
                nc.tensor.matmul(bcp, lhsT=gmaskT_t, rhs=grp2[:, ck, :],
                                 start=True, stop=True)
                nc.vector.tensor_mul(ab[:, ck, 0:1], bcp[:, 1:2],
                                     gns_t[:, ck:ck + 1])
                tmpb = stat.tile([128, 1], F32, tag="tmpb")
                nc.vector.tensor_mul(tmpb, bcp[:, 0:1], ab[:, ck, 0:1])
                nc.vector.tensor_sub(ab[:, ck, 1:2], gnb_t[:, ck:ck + 1], tmpb)
                # xn = x*A + B   (bf16)
                nc.vector.tensor_scalar(xn_t[:, ck, :], x_t[:, ck, :],
                                        ab[:, ck, 0:1], ab[:, ck, 1:2],
                                        op0=MULT, op1=ADD)
            # fold b_proj into the residual source: x_t += b_proj (per channel)
            for ck in range(NCK):
                nc.vector.tensor_scalar_add(x_t[:, ck, :], x_t[:, ck, :],
                                            bp_t[:, ck:ck + 1])

            # ---- QKV: v transposed (vT[t, h*65+j], col h*65+64 == 1.0) ----
            vT_t = vp.tile([128, 8, HEADS * 65], BF16, tag="vT")
            for tcn in range(8):
                psv = psA.tile([128, 2, 512], F32, tag="big")
                for ns in range(2):
                    for kc in range(NCK):
                        nc.tensor.matmul(
                            psv[:, ns, 0:260],
                            lhsT=xn_t[:, kc, tcn * 128:(tcn + 1) * 128],
                            rhs=wvT_t[:, kc, ns * 260:(ns + 1) * 260],
                            start=(kc == 0), stop=(kc == NCK - 1))
                nc.vector.tensor_add(vT_t[:, tcn, :], psv[:, :, 0:260], bv_t)

            # ---- per-head: qk matmul, scores, softmax, av ----
            a_t = ap_.tile([128, NCK, T], BF16, tag="a")
            for h in range(HEADS):
                # q,k for head h: [128(q|k), T] psum -> split to base-0 tiles
                psqk = psA.tile([128, 1024], F32, tag="big")
                for nt in range(2):
                    for kc in range(NCK):
                        nc.tensor.matmul(
                            psqk[:, nt * 512:(nt + 1) * 512],
                            lhsT=wqkT_t[:, kc, h * 128:(h + 1) * 128],
                            rhs=xn_t[:, kc, nt * 512:(nt + 1) * 512],
                            start=(kc == 0), stop=(kc == NCK - 1))
                q_t = qkp.tile([64, T], BF16, tag="q")
                k_t = qkp.tile([64, T], BF16, tag="k")
                nc.vector.tensor_scalar_add(q_t, psqk[0:64, :], bqk_t[0:64, h:h + 1])
                nc.vector.tensor_scalar_add(k_t, psqk[64:128, :], bqk_t[64:128, h:h + 1])

                # scores transposed: wT[s, t] = sum_c k[c,s] q[c,t]; exp fused
                pT_t = pp.tile([128, 8, T], BF16, tag="pT")
                for sc in range(8):
                    psw = psA.tile([128, 1024], F32, tag="big")
                    for nt in range(2):
                        nc.tensor.matmul(
                            psw[:, nt * 512:(nt + 1) * 512],
                            lhsT=k_t[:, sc * 128:(sc + 1) * 128],
                            rhs=q_t[:, nt * 512:(nt + 1) * 512],
                            start=True, stop=True)
                    nc.scalar.activation(pT_t[:, sc, :], psw, Exp, scale=SCALE2)

                # av: [65, T]; row 64 = Z (ones column of vT)
                psa = psB.tile([65, 1024], F32, tag="av")
                for sc in range(8):
                    for nt in range(2):
                        nc.tensor.matmul(
                            psa[:, nt * 512:(nt + 1) * 512],
                            lhsT=vT_t[:, sc, h * 65:(h + 1) * 65],
                            rhs=pT_t[:, sc, nt * 512:(nt + 1) * 512],
                            start=(sc == 0), stop=(sc == 7))
                rz = rzp.tile([1, T], F32, tag="rz")
                nc.vector.reciprocal(rz, psa[64:65, :])
                rzb = rzp.tile([64, T], F32, tag="rzb")
                nc.gpsimd.partition_broadcast(rzb, rz)
                # a[j, t] = av[j, t] / Z[t]  -> packed [c=h*64+j] layout
                pbase = (h % 2) * 64
                nc.vector.tensor_mul(
                    a_t[pbase:pbase + 64, h // 2, :], psa[0:64, :], rzb)

            # ---- proj + bias + residual ----
            for oc in range(NCK):
                psh = psA.tile([128, 2, 512], F32, tag="big")
                for nt in range(2):
                    for kc in range(NCK):
                        nc.tensor.matmul(
                            psh[:, nt, :],
                            lhsT=wpT_t[:, kc, oc * 128:(oc + 1) * 128],
                            rhs=a_t[:, kc, nt * 512:(nt + 1) * 512],
                            start=(kc == 0), stop=(kc == NCK - 1))
                o_t = op.tile([128, T], F32, tag="o")
                nc.vector.tensor_add(o_t, psh, x_t[:, oc, :])
                nc.sync.dma_start(
                    out_d[b].rearrange("(o p) t -> p o t", p=128)[:, oc, :], o_t)

    nc.compile()
    return nc


def _pack_inputs(inputs):
    """Host-side packing: shard x over batch, pre-transpose/pack weights."""
    x = np.ascontiguousarray(np.asarray(inputs["x"], dtype=np.float32)).reshape(B, C, T)
    gn_scale = np.asarray(inputs["gn_scale"], dtype=np.float32)
    gn_bias = np.asarray(inputs["gn_bias"], dtype=np.float32)
    w_qkv = np.asarray(inputs["w_qkv"], dtype=np.float32)
    b_qkv = np.asarray(inputs["b_qkv"], dtype=np.float32)
    w_proj = np.asarray(inputs["w_proj"], dtype=np.float32)
    b_proj = np.asarray(inputs["b_proj"], dtype=np.float32)

    # per-head qk rows (h*192 .. h*192+128) and v rows (h*192+128 .. h*192+192)
    wqk = np.empty((C, HEADS * 128), np.float32)
    bqk = np.empty((128, HEADS), np.float32)
    wv = np.zeros((C, HEADS * 65), np.float32)
    bv = np.zeros((HEADS * 65,), np.float32)
    for h in range(HEADS):
        r0 = h * 3 * CH
        wqk[:, h * 128:(h + 1) * 128] = w_qkv[r0:r0 + 128, :].T
        bqk[:, h] = b_qkv[r0:r0 + 128]
        wv[:, h * 65:h * 65 + 64] = w_qkv[r0 + 128:r0 + 192, :].T
        bv[h * 65:h * 65 + 64] = b_qkv[r0 + 128:r0 + 192]
        bv[h * 65 + 64] = 1.0

    gmask = np.zeros((128, 8), np.float32)
    for g in range(8):
        gmask[g * 16:(g + 1) * 16, g] = 1.0

    common = {
        "wqkT": wqk.astype(ml_dtypes.bfloat16),
        "wvT": wv.astype(ml_dtypes.bfloat16),
        "wpT": np.ascontiguousarray(w_proj.T).astype(ml_dtypes.bfloat16),
        "bqk": bqk,
        "bv": bv,
        "bp": np.ascontiguousarray(b_proj.reshape(NCK, 128).T),
        "gns": np.ascontiguousarray(gn_scale.reshape(NCK, 128).T),
        "gnb": np.ascontiguousarray(gn_bias.reshape(NCK, 128).T),
        "gmask": gmask,
        "gmaskT": np.ascontiguousarray(gmask.T),
    }
    in_maps = []
    for i in range(NCORES):
        m = dict(common)
        m["x"] = np.ascontiguousarray(x[i * NB:(i + 1) * NB])
        in_maps.append(m)
    return in_maps


_NC_CACHE = None


def _get_nc():
    global _NC_CACHE
    if _NC_CACHE is None:
        _NC_CACHE = build_graph()
    return _NC_CACHE


def _run(inputs, trace=False, **kwargs):
    nc = _get_nc()
    in_maps = _pack_inputs(inputs)
    res = run_bass_kernel_spmd(nc, in_maps, core_ids=list(range(NCORES)),
                               trace=trace, **kwargs)
    outs = np.concatenate([np.asarray(res.results[i]["out"], dtype=np.float32)
                           for i in range(NCORES)], axis=0)
    return outs.reshape(B, C, 32, 32), res


def kernel(**inputs) -> np.ndarray:
    out, _ = _run(inputs, trace=False)
    return out


# revision 13
# speedup vs baseline: 1.1067x; 1.1067x over previous
"""Trainium2 Bass kernel for nn_AttentionBlock (GroupNorm -> QKV -> MHA -> proj -> residual).

Sharding: pure data-parallel over batch. B=16 across 8 cores -> 2 batches/core.
No collectives needed. Weights replicated, host pre-transposed/packed.

Self-contained: hardcodes shapes B=16, C=512, H=W=32 (T=1024), HEADS=8, GROUPS=32.
"""
import sys
import os

for _p in ("/opt/trn_rl_repo", "/root/.axon_site/_ro/trn_rl_repo"):
    if os.path.isdir(_p) and _p not in sys.path:
        sys.path.insert(0, _p)

import numpy as np
import ml_dtypes
from contextlib import ExitStack

import concourse.bass as bass
import concourse.tile as tile
from concourse import mybir, bacc
from concourse.bass_utils import run_bass_kernel_spmd

F32 = mybir.dt.float32
BF16 = mybir.dt.bfloat16

NCORES = 8
B = 16
C = 512
T = 1024
HEADS = 8
GROUPS = 32
CH = C // HEADS          # 64
NB = B // NCORES         # 2 batches per core
NCK = C // 128           # 4 channel chunks
EPS = 1e-5
SCALE2 = float(CH) ** -0.5  # (ch^-0.25)^2 folded into exp scale


def build_graph():
    nc = bacc.Bacc("TRN2", target_bir_lowering=False, debug=False,
                   num_devices=NCORES)

    x_d = nc.dram_tensor("x", [NB, C, T], F32, kind="ExternalInput").ap()
    wqkT_d = nc.dram_tensor("wqkT", [C, HEADS * 128], BF16, kind="ExternalInput").ap()
    wvT_d = nc.dram_tensor("wvT", [C, HEADS * 65], BF16, kind="ExternalInput").ap()
    wpT_d = nc.dram_tensor("wpT", [C, C], BF16, kind="ExternalInput").ap()
    bqk_d = nc.dram_tensor("bqk", [128, HEADS], F32, kind="ExternalInput").ap()
    bv_d = nc.dram_tensor("bv", [HEADS * 65], F32, kind="ExternalInput").ap()
    bp_d = nc.dram_tensor("bp", [128, NCK], F32, kind="ExternalInput").ap()
    gns_d = nc.dram_tensor("gns", [128, NCK], F32, kind="ExternalInput").ap()
    gnb_d = nc.dram_tensor("gnb", [128, NCK], F32, kind="ExternalInput").ap()
    gmask_d = nc.dram_tensor("gmask", [128, 8], F32, kind="ExternalInput").ap()
    gmaskT_d = nc.dram_tensor("gmaskT", [8, 128], F32, kind="ExternalInput").ap()
    out_d = nc.dram_tensor("out", [NB, C, T], F32, kind="ExternalOutput").ap()

    MULT = mybir.AluOpType.mult
    ADD = mybir.AluOpType.add
    SUB = mybir.AluOpType.subtract
    Exp = mybir.ActivationFunctionType.Exp
    Sqrt = mybir.ActivationFunctionType.Sqrt

    with tile.TileContext(nc) as tc, ExitStack() as ctx:
        consts = ctx.enter_context(tc.tile_pool(name="consts", bufs=1))
        xp = ctx.enter_context(tc.tile_pool(name="xp", bufs=2))
        xnp = ctx.enter_context(tc.tile_pool(name="xnp", bufs=2))
        qkp = ctx.enter_context(tc.tile_pool(name="qkp", bufs=4))
        vp = ctx.enter_context(tc.tile_pool(name="vp", bufs=2))
        pp = ctx.enter_context(tc.tile_pool(name="pp", bufs=2))
        ap_ = ctx.enter_context(tc.tile_pool(name="ap", bufs=2))
        op = ctx.enter_context(tc.tile_pool(name="op", bufs=4))
        stat = ctx.enter_context(tc.tile_pool(name="stat", bufs=2))
        rzp = ctx.enter_context(tc.tile_pool(name="rzp", bufs=2))
        psA = ctx.enter_context(tc.tile_pool(name="psA", bufs=4, space="PSUM"))

        # ---- constants (loaded once) ----
        wqkT_t = consts.tile([128, NCK, HEADS * 128], BF16)
        nc.sync.dma_start(wqkT_t, wqkT_d.rearrange("(o p) n -> p o n", p=128))
        wvT_t = consts.tile([128, NCK, HEADS * 65], BF16)
        nc.sync.dma_start(wvT_t, wvT_d.rearrange("(o p) n -> p o n", p=128))
        wpT_t = consts.tile([128, NCK, C], BF16)
        nc.sync.dma_start(wpT_t, wpT_d.rearrange("(o p) n -> p o n", p=128))
        bqk_t = consts.tile([128, HEADS], F32)
        nc.sync.dma_start(bqk_t, bqk_d)
        bv_t = consts.tile([128, HEADS * 65], F32)
        nc.sync.dma_start(bv_t, bv_d[None, :].to_broadcast([128, HEADS * 65]))
        bp_t = consts.tile([128, NCK], F32)
        nc.sync.dma_start(bp_t, bp_d)
        gns_t = consts.tile([128, NCK], F32)
        nc.sync.dma_start(gns_t, gns_d)
        gnb_t = consts.tile([128, NCK], F32)
        nc.sync.dma_start(gnb_t, gnb_d)
        gmask_t = consts.tile([128, 8], F32)
        nc.sync.dma_start(gmask_t, gmask_d)
        gmaskT_t = consts.tile([8, 128], F32)
        nc.sync.dma_start(gmaskT_t, gmaskT_d)
        eps_t = consts.tile([8, 1], F32)
        nc.vector.memset(eps_t, EPS)

        for b in range(NB):
            # ---- load x ----
            x_t = xp.tile([128, NCK, T], F32, tag="x")
            nc.sync.dma_start(x_t, x_d[b].rearrange("(o p) t -> p o t", p=128))

            # ---- GroupNorm stats ----
            mv = stat.tile([128, NCK, 2], F32, tag="mv")
            for ck in range(NCK):
                st = stat.tile([128, 2, 6], F32, tag="bnst")
                nc.vector.bn_stats(st[:, 0, :], x_t[:, ck, 0:512])
                nc.vector.bn_stats(st[:, 1, :], x_t[:, ck, 512:1024])
                nc.vector.bn_aggr(mv[:, ck, :], st)
            # me2: col0 = mean_c, col1 = E[x^2]_c = var_c + mean_c^2
            me2 = stat.tile([128, NCK, 2], F32, tag="me2")
            sq = stat.tile([128, NCK, 1], F32, tag="sq")
            nc.vector.tensor_copy(me2[:, :, 0:1], mv[:, :, 0:1])
            nc.vector.tensor_mul(sq, mv[:, :, 0:1], mv[:, :, 0:1])
            nc.vector.tensor_add(me2[:, :, 1:2], mv[:, :, 1:2], sq)

            # group aggregation: per chunk [8 groups, 2] = gmask.T @ me2 ; /16
            st8 = stat.tile([8, NCK, 2], F32, tag="st8")
            for ck in range(NCK):
                gps = psA.tile([8, 2], F32, tag="big")
                nc.tensor.matmul(gps, lhsT=gmask_t, rhs=me2[:, ck, :],
                                 start=True, stop=True)
                nc.vector.tensor_scalar_mul(st8[:, ck, :], gps, 1.0 / 16.0)
            # rstd_g = 1/sqrt(var_g + eps); var_g = E2_g - mean_g^2
            msq8 = stat.tile([8, NCK, 1], F32, tag="msq8")
            nc.vector.tensor_mul(msq8, st8[:, :, 0:1], st8[:, :, 0:1])
            var8 = stat.tile([8, NCK, 1], F32, tag="var8")
            nc.vector.tensor_sub(var8, st8[:, :, 1:2], msq8)
            sd8 = stat.tile([8, NCK, 1], F32, tag="sd8")
            nc.scalar.activation(sd8, var8, Sqrt, bias=eps_t)
            grp2 = stat.tile([8, NCK, 2], F32, tag="grp2")
            nc.vector.reciprocal(grp2[:, :, 1:2], sd8)
            nc.vector.tensor_copy(grp2[:, :, 0:1], st8[:, :, 0:1])

            # broadcast group stats back to channels; A = rstd*gamma, B = beta - mean*A
            ab = stat.tile([128, NCK, 2], F32, tag="ab")
            xn_t = xnp.tile([128, NCK, T], BF16, tag="xn")
            for ck in range(NCK):
                bcp = psA.tile([128, 2], F32, tag="big")
                nc.tensor.matmul(bcp, lhsT=gmaskT_t, rhs=grp2[:, ck, :],
                                 start=True, stop=True)
                nc.vector.tensor_mul(ab[:, ck, 0:1], bcp[:, 1:2],
                                     gns_t[:, ck:ck + 1])
                tmpb = stat.tile([128, 1], F32, tag="tmpb")
                nc.vector.tensor_mul(tmpb, bcp[:, 0:1], ab[:, ck, 0:1])
                nc.vector.tensor_sub(ab[:, ck, 1:2], gnb_t[:, ck:ck + 1], tmpb)
                # xn = x*A + B   (bf16)
                nc.vector.tensor_scalar(xn_t[:, ck, :], x_t[:, ck, :],
                                        ab[:, ck, 0:1], ab[:, ck, 1:2],
                                        op0=MULT, op1=ADD)


            # ---- QKV: v transposed (vT[t, h*65+j], col h*65+64 == 1.0) ----
            vT_t = vp.tile([128, 8, HEADS * 65], BF16, tag="vT")
            for tcn in range(8):
                psv = psA.tile([128, 2, 512], F32, tag="big")
                for ns in range(2):
                    for kc in range(NCK):
                        nc.tensor.matmul(
                            psv[:, ns, 0:260],
                            lhsT=xn_t[:, kc, tcn * 128:(tcn + 1) * 128],
                            rhs=wvT_t[:, kc, ns * 260:(ns + 1) * 260],
                            start=(kc == 0), stop=(kc == NCK - 1))
                nc.vector.tensor_add(vT_t[:, tcn, :], psv[:, :, 0:260], bv_t)

            # ---- per-head: qk matmul, scores, softmax, av ----
            a_t = ap_.tile([128, NCK, T], BF16, tag="a")
            for h in range(HEADS):
                # q,k for head h: [128(q|k), T] psum -> split to base-0 tiles
                psqk = psA.tile([128, 1024], F32, tag="big")
                for nt in range(2):
                    for kc in range(NCK):
                        nc.tensor.matmul(
                            psqk[:, nt * 512:(nt + 1) * 512],
                            lhsT=wqkT_t[:, kc, h * 128:(h + 1) * 128],
                            rhs=xn_t[:, kc, nt * 512:(nt + 1) * 512],
                            start=(kc == 0), stop=(kc == NCK - 1))
                q_t = qkp.tile([64, T], BF16, tag="q")
                k_t = qkp.tile([64, T], BF16, tag="k")
                nc.vector.tensor_scalar_add(q_t, psqk[0:64, :], bqk_t[0:64, h:h + 1])
                nc.vector.tensor_scalar_add(k_t, psqk[64:128, :], bqk_t[64:128, h:h + 1])

                # scores transposed: wT[s, t] = sum_c k[c,s] q[c,t]; exp fused
                pT_t = pp.tile([128, 8, T], BF16, tag="pT")
                for sc in range(8):
                    psw = psA.tile([128, 1024], F32, tag="big")
                    for nt in range(2):
                        nc.tensor.matmul(
                            psw[:, nt * 512:(nt + 1) * 512],
                            lhsT=k_t[:, sc * 128:(sc + 1) * 128],
                            rhs=q_t[:, nt * 512:(nt + 1) * 512],
                            start=True, stop=True)
                    nc.scalar.activation(pT_t[:, sc, :], psw, Exp, scale=SCALE2)

                # av: [65, T]; row 64 = Z (ones column of vT)
                psa = psA.tile([65, 1024], F32, tag="big")
                for sc in range(8):
                    for nt in range(2):
                        nc.tensor.matmul(
                            psa[:, nt * 512:(nt + 1) * 512],
                            lhsT=vT_t[:, sc, h * 65:(h + 1) * 65],
                            rhs=pT_t[:, sc, nt * 512:(nt + 1) * 512],
                            start=(sc == 0), stop=(sc == 7))
                rz = rzp.tile([1, T], F32, tag="rz")
                nc.vector.reciprocal(rz, psa[64:65, :])
                rzb = rzp.tile([64, T], F32, tag="rzb")
                nc.gpsimd.partition_broadcast(rzb, rz)
                # a[j, t] = av[j, t] / Z[t]  -> packed [c=h*64+j] layout
                pbase = (h % 2) * 64
                nc.vector.tensor_mul(
                    a_t[pbase:pbase + 64, h // 2, :], psa[0:64, :], rzb)

            # ---- proj + bias + residual ----
            for oc in range(NCK):
                psh = psA.tile([128, 2, 512], F32, tag="big")
                for nt in range(2):
                    for kc in range(NCK):
                        nc.tensor.matmul(
                            psh[:, nt, :],
                            lhsT=wpT_t[:, kc, oc * 128:(oc + 1) * 128],
                            rhs=a_t[:, kc, nt * 512:(nt + 1) * 512],
                            start=(kc == 0), stop=(kc == NCK - 1))
                o_t = op.tile([128, T], F32, tag="o")
                # o = (psh + b_proj) + x  in one DVE pass
                nc.vector.scalar_tensor_tensor(
                    o_t, psh, bp_t[:, oc:oc + 1], x_t[:, oc, :],
                    op0=ADD, op1=ADD)
                nc.sync.dma_start(
                    out_d[b].rearrange("(o p) t -> p o t", p=128)[:, oc, :], o_t)

    nc.compile()
    return nc


def _pack_inputs(inputs):
    """Host-side packing: shard x over batch, pre-transpose/pack weights."""
    x = np.ascontiguousarray(np.asarray(inputs["x"], dtype=np.float32)).reshape(B, C, T)
    gn_scale = np.asarray(inputs["gn_scale"], dtype=np.float32)
    gn_bias = np.asarray(inputs["gn_bias"], dtype=np.float32)
    w_qkv = np.asarray(inputs["w_qkv"], dtype=np.float32)
    b_qkv = np.asarray(inputs["b_qkv"], dtype=np.float32)
    w_proj = np.asarray(inputs["w_proj"], dtype=np.float32)
    b_proj = np.asarray(inputs["b_proj"], dtype=np.float32)

    # per-head qk rows (h*192 .. h*192+128) and v rows (h*192+128 .. h*192+192)
    wqk = np.empty((C, HEADS * 128), np.float32)
    bqk = np.empty((128, HEADS), np.float32)
    wv = np.zeros((C, HEADS * 65), np.float32)
    bv = np.zeros((HEADS * 65,), np.float32)
    for h in range(HEADS):
        r0 = h * 3 * CH
        wqk[:, h * 128:(h + 1) * 128] = w_qkv[r0:r0 + 128, :].T
        bqk[:, h] = b_qkv[r0:r0 + 128]
        wv[:, h * 65:h * 65 + 64] = w_qkv[r0 + 128:r0 + 192, :].T
        bv[h * 65:h * 65 + 64] = b_qkv[r0 + 128:r0 + 192]
        bv[h * 65 + 64] = 1.0

    gmask = np.zeros((128, 8), np.float32)
    for g in range(8):
        gmask[g * 16:(g + 1) * 16, g] = 1.0

    common = {
        "wqkT": wqk.astype(ml_dtypes.bfloat16),
        "wvT": wv.astype(ml_dtypes.bfloat16),
        "wpT": np.ascontiguousarray(w_proj.T).astype(ml_dtypes.bfloat16),
        "bqk": bqk,
        "bv": bv,
        "bp": np.ascontiguousarray(b_proj.reshape(NCK, 128).T),
        "gns": np.ascontiguousarray(gn_scale.reshape(NCK, 128).T),
        "gnb": np.ascontiguousarray(gn_bias.reshape(NCK, 128).T),
        "gmask": gmask,
        "gmaskT": np.ascontiguousarray(gmask.T),
    }
    in_maps = []
    for i in range(NCORES):
        m = dict(common)
        m["x"] = np.ascontiguousarray(x[i * NB:(i + 1) * NB])
        in_maps.append(m)
    return in_maps


_NC_CACHE = None


def _get_nc():
    global _NC_CACHE
    if _NC_CACHE is None:
        _NC_CACHE = build_graph()
    return _NC_CACHE


def _run(inputs, trace=False, **kwargs):
    nc = _get_nc()
    in_maps = _pack_inputs(inputs)
    res = run_bass_kernel_spmd(nc, in_maps, core_ids=list(range(NCORES)),
                               trace=trace, **kwargs)
    outs = np.concatenate([np.asarray(res.results[i]["out"], dtype=np.float32)
                           for i in range(NCORES)], axis=0)
    return outs.reshape(B, C, 32, 32), res


def kernel(**inputs) -> np.ndarray:
    out, _ = _run(inputs, trace=False)
    return out


# revision 14
# speedup vs baseline: 1.1675x; 1.0549x over previous
"""Trainium2 Bass kernel for nn_AttentionBlock (GroupNorm -> QKV -> MHA -> proj -> residual).

Sharding: pure data-parallel over batch. B=16 across 8 cores -> 2 batches/core.
No collectives needed. Weights replicated, host pre-transposed/packed.

Self-contained: hardcodes shapes B=16, C=512, H=W=32 (T=1024), HEADS=8, GROUPS=32.
"""
import sys
import os

for _p in ("/opt/trn_rl_repo", "/root/.axon_site/_ro/trn_rl_repo"):
    if os.path.isdir(_p) and _p not in sys.path:
        sys.path.insert(0, _p)

import numpy as np
import ml_dtypes
from contextlib import ExitStack

import concourse.bass as bass
import concourse.tile as tile
from concourse import mybir, bacc
from concourse.bass_utils import run_bass_kernel_spmd

F32 = mybir.dt.float32
BF16 = mybir.dt.bfloat16

NCORES = 8
B = 16
C = 512
T = 1024
HEADS = 8
GROUPS = 32
CH = C // HEADS          # 64
NB = B // NCORES         # 2 batches per core
NCK = C // 128           # 4 channel chunks
EPS = 1e-5
SCALE2 = float(CH) ** -0.5  # (ch^-0.25)^2 folded into exp scale


def build_graph():
    nc = bacc.Bacc("TRN2", target_bir_lowering=False, debug=False,
                   num_devices=NCORES)

    x_d = nc.dram_tensor("x", [NB, C, T], F32, kind="ExternalInput").ap()
    wqkT_d = nc.dram_tensor("wqkT", [C, HEADS * 128], BF16, kind="ExternalInput").ap()
    wvT_d = nc.dram_tensor("wvT", [C, HEADS * 65], BF16, kind="ExternalInput").ap()
    wpT_d = nc.dram_tensor("wpT", [C, C], BF16, kind="ExternalInput").ap()
    bqk_d = nc.dram_tensor("bqk", [128, HEADS], F32, kind="ExternalInput").ap()
    bv_d = nc.dram_tensor("bv", [HEADS * 65], F32, kind="ExternalInput").ap()
    bp_d = nc.dram_tensor("bp", [128, NCK], F32, kind="ExternalInput").ap()
    gns_d = nc.dram_tensor("gns", [128, NCK], F32, kind="ExternalInput").ap()
    gnb_d = nc.dram_tensor("gnb", [128, NCK], F32, kind="ExternalInput").ap()
    gmask_d = nc.dram_tensor("gmask", [128, 8], F32, kind="ExternalInput").ap()
    gmaskT_d = nc.dram_tensor("gmaskT", [8, 128], F32, kind="ExternalInput").ap()
    out_d = nc.dram_tensor("out", [NB, C, T], F32, kind="ExternalOutput").ap()

    MULT = mybir.AluOpType.mult
    ADD = mybir.AluOpType.add
    SUB = mybir.AluOpType.subtract
    Exp = mybir.ActivationFunctionType.Exp
    Sqrt = mybir.ActivationFunctionType.Sqrt

    with tile.TileContext(nc) as tc, ExitStack() as ctx:
        consts = ctx.enter_context(tc.tile_pool(name="consts", bufs=1))
        xp = ctx.enter_context(tc.tile_pool(name="xp", bufs=2))
        xnp = ctx.enter_context(tc.tile_pool(name="xnp", bufs=2))
        qkp = ctx.enter_context(tc.tile_pool(name="qkp", bufs=4))
        vp = ctx.enter_context(tc.tile_pool(name="vp", bufs=2))
        pp = ctx.enter_context(tc.tile_pool(name="pp", bufs=2))
        ap_ = ctx.enter_context(tc.tile_pool(name="ap", bufs=2))
        op = ctx.enter_context(tc.tile_pool(name="op", bufs=4))
        stat = ctx.enter_context(tc.tile_pool(name="stat", bufs=2))
        rzp = ctx.enter_context(tc.tile_pool(name="rzp", bufs=2))
        psA = ctx.enter_context(tc.tile_pool(name="psA", bufs=4, space="PSUM"))

        # ---- constants (loaded once) ----
        wqkT_t = consts.tile([128, NCK, HEADS * 128], BF16)
        nc.sync.dma_start(wqkT_t, wqkT_d.rearrange("(o p) n -> p o n", p=128))
        wvT_t = consts.tile([128, NCK, HEADS * 65], BF16)
        nc.sync.dma_start(wvT_t, wvT_d.rearrange("(o p) n -> p o n", p=128))
        wpT_t = consts.tile([128, NCK, C], BF16)
        nc.sync.dma_start(wpT_t, wpT_d.rearrange("(o p) n -> p o n", p=128))
        bqk_t = consts.tile([128, HEADS], F32)
        nc.sync.dma_start(bqk_t, bqk_d)
        bv_t = consts.tile([128, HEADS * 65], F32)
        nc.sync.dma_start(bv_t, bv_d[None, :].to_broadcast([128, HEADS * 65]))
        bp_t = consts.tile([128, NCK], F32)
        nc.sync.dma_start(bp_t, bp_d)
        gns_t = consts.tile([128, NCK], F32)
        nc.sync.dma_start(gns_t, gns_d)
        gnb_t = consts.tile([128, NCK], F32)
        nc.sync.dma_start(gnb_t, gnb_d)
        gmask_t = consts.tile([128, 8], F32)
        nc.sync.dma_start(gmask_t, gmask_d)
        gmaskT_t = consts.tile([8, 128], F32)
        nc.sync.dma_start(gmaskT_t, gmaskT_d)
        eps_t = consts.tile([8, 1], F32)
        nc.vector.memset(eps_t, EPS)

        for b in range(NB):
            # ---- load x ----
            x_t = xp.tile([128, NCK, T], F32, tag="x")
            nc.sync.dma_start(x_t, x_d[b].rearrange("(o p) t -> p o t", p=128))

            # ---- GroupNorm stats ----
            mv = stat.tile([128, NCK, 2], F32, tag="mv")
            for ck in range(NCK):
                st = stat.tile([128, 2, 6], F32, tag="bnst")
                nc.vector.bn_stats(st[:, 0, :], x_t[:, ck, 0:512])
                nc.vector.bn_stats(st[:, 1, :], x_t[:, ck, 512:1024])
                nc.vector.bn_aggr(mv[:, ck, :], st)
            # me2: col0 = mean_c, col1 = E[x^2]_c = var_c + mean_c^2
            me2 = stat.tile([128, NCK, 2], F32, tag="me2")
            sq = stat.tile([128, NCK, 1], F32, tag="sq")
            nc.vector.tensor_copy(me2[:, :, 0:1], mv[:, :, 0:1])
            nc.vector.tensor_mul(sq, mv[:, :, 0:1], mv[:, :, 0:1])
            nc.vector.tensor_add(me2[:, :, 1:2], mv[:, :, 1:2], sq)

            # group aggregation: per chunk [8 groups, 2] = gmask.T @ me2 ; /16
            st8 = stat.tile([8, NCK, 2], F32, tag="st8")
            for ck in range(NCK):
                gps = psA.tile([8, 2], F32, tag="big")
                nc.tensor.matmul(gps, lhsT=gmask_t, rhs=me2[:, ck, :],
                                 start=True, stop=True)
                nc.vector.tensor_scalar_mul(st8[:, ck, :], gps, 1.0 / 16.0)
            # rstd_g = 1/sqrt(var_g + eps); var_g = E2_g - mean_g^2
            msq8 = stat.tile([8, NCK, 1], F32, tag="msq8")
            nc.vector.tensor_mul(msq8, st8[:, :, 0:1], st8[:, :, 0:1])
            var8 = stat.tile([8, NCK, 1], F32, tag="var8")
            nc.vector.tensor_sub(var8, st8[:, :, 1:2], msq8)
            sd8 = stat.tile([8, NCK, 1], F32, tag="sd8")
            nc.scalar.activation(sd8, var8, Sqrt, bias=eps_t)
            grp2 = stat.tile([8, NCK, 2], F32, tag="grp2")
            nc.vector.reciprocal(grp2[:, :, 1:2], sd8)
            nc.vector.tensor_copy(grp2[:, :, 0:1], st8[:, :, 0:1])

            # broadcast group stats back to channels; A = rstd*gamma, B = beta - mean*A
            ab = stat.tile([128, NCK, 2], F32, tag="ab")
            xn_t = xnp.tile([128, NCK, T], BF16, tag="xn")
            for ck in range(NCK):
                bcp = psA.tile([128, 2], F32, tag="big")
                nc.tensor.matmul(bcp, lhsT=gmaskT_t, rhs=grp2[:, ck, :],
                                 start=True, stop=True)
                nc.vector.tensor_mul(ab[:, ck, 0:1], bcp[:, 1:2],
                                     gns_t[:, ck:ck + 1])
                tmpb = stat.tile([128, 1], F32, tag="tmpb")
                nc.vector.tensor_mul(tmpb, bcp[:, 0:1], ab[:, ck, 0:1])
                nc.vector.tensor_sub(ab[:, ck, 1:2], gnb_t[:, ck:ck + 1], tmpb)
                # xn = x*A + B   (bf16)
                nc.vector.tensor_scalar(xn_t[:, ck, :], x_t[:, ck, :],
                                        ab[:, ck, 0:1], ab[:, ck, 1:2],
                                        op0=MULT, op1=ADD)


            # ---- QKV: v transposed (vT[t, h*65+j], col h*65+64 == 1.0) ----
            vT_t = vp.tile([128, 8, HEADS * 65], BF16, tag="vT")
            for tcn in range(8):
                psv = psA.tile([128, 2, 512], F32, tag="big")
                for ns in range(2):
                    for kc in range(NCK):
                        nc.tensor.matmul(
                            psv[:, ns, 0:260],
                            lhsT=xn_t[:, kc, tcn * 128:(tcn + 1) * 128],
                            rhs=wvT_t[:, kc, ns * 260:(ns + 1) * 260],
                            start=(kc == 0), stop=(kc == NCK - 1))
                nc.vector.tensor_add(vT_t[:, tcn, :], psv[:, :, 0:260], bv_t)

            # ---- per-head: qk matmul, scores, softmax, av ----
            a_t = ap_.tile([128, NCK, T], BF16, tag="a")
            for h in range(HEADS):
                # q,k for head h: [128(q|k), T] psum -> split to base-0 tiles
                psqk = psA.tile([128, 1024], F32, tag="big")
                for nt in range(2):
                    for kc in range(NCK):
                        nc.tensor.matmul(
                            psqk[:, nt * 512:(nt + 1) * 512],
                            lhsT=wqkT_t[:, kc, h * 128:(h + 1) * 128],
                            rhs=xn_t[:, kc, nt * 512:(nt + 1) * 512],
                            start=(kc == 0), stop=(kc == NCK - 1))
                q_t = qkp.tile([64, T], BF16, tag="q")
                k_t = qkp.tile([64, T], BF16, tag="k")
                nc.vector.tensor_scalar_add(q_t, psqk[0:64, :], bqk_t[0:64, h:h + 1])
                nc.vector.tensor_scalar_add(k_t, psqk[64:128, :], bqk_t[64:128, h:h + 1])

                # scores transposed: wT[s, t] = sum_c k[c,s] q[c,t]; exp fused.
                # Interleaved with the av accumulation so the PE alternates
                # scores(sc+1) with av(sc) while ACT runs exp(sc).
                pT_t = pp.tile([128, 8, T], BF16, tag="pT")
                psa = psA.tile([65, 1024], F32, tag="big")
                for sc in range(8):
                    psw = psA.tile([128, 1024], F32, tag="big")
                    for nt in range(2):
                        nc.tensor.matmul(
                            psw[:, nt * 512:(nt + 1) * 512],
                            lhsT=k_t[:, sc * 128:(sc + 1) * 128],
                            rhs=q_t[:, nt * 512:(nt + 1) * 512],
                            start=True, stop=True)
                    nc.scalar.activation(pT_t[:, sc, :], psw, Exp, scale=SCALE2)
                    for nt in range(2):
                        nc.tensor.matmul(
                            psa[:, nt * 512:(nt + 1) * 512],
                            lhsT=vT_t[:, sc, h * 65:(h + 1) * 65],
                            rhs=pT_t[:, sc, nt * 512:(nt + 1) * 512],
                            start=(sc == 0), stop=(sc == 7))
                # Z = psa row 64; 1/Z via fast recip (SBUF-only op, so copy out
                # of PSUM on the scalar engine first)
                zrow = rzp.tile([1, T], F32, tag="zrow")
                nc.scalar.copy(zrow, psa[64:65, :])
                rz = rzp.tile([1, T], F32, tag="rz")
                nc.vector.reciprocal_approx_fast(rz, zrow)
                rzb = rzp.tile([64, T], F32, tag="rzb")
                nc.gpsimd.partition_broadcast(rzb, rz)
                # a[j, t] = av[j, t] / Z[t]  -> packed [c=h*64+j] layout
                pbase = (h % 2) * 64
                nc.vector.tensor_mul(
                    a_t[pbase:pbase + 64, h // 2, :], psa[0:64, :], rzb)

            # ---- proj + bias + residual ----
            for oc in range(NCK):
                psh = psA.tile([128, 2, 512], F32, tag="big")
                for nt in range(2):
                    for kc in range(NCK):
                        nc.tensor.matmul(
                            psh[:, nt, :],
                            lhsT=wpT_t[:, kc, oc * 128:(oc + 1) * 128],
                            rhs=a_t[:, kc, nt * 512:(nt + 1) * 512],
                            start=(kc == 0), stop=(kc == NCK - 1))
                o_t = op.tile([128, T], F32, tag="o")
                # o = (psh + b_proj) + x  in one DVE pass
                nc.vector.scalar_tensor_tensor(
                    o_t, psh, bp_t[:, oc:oc + 1], x_t[:, oc, :],
                    op0=ADD, op1=ADD)
                nc.sync.dma_start(
                    out_d[b].rearrange("(o p) t -> p o t", p=128)[:, oc, :], o_t)

    nc.compile()
    return nc


def _pack_inputs(inputs):
    """Host-side packing: shard x over batch, pre-transpose/pack weights."""
    x = np.ascontiguousarray(np.asarray(inputs["x"], dtype=np.float32)).reshape(B, C, T)
    gn_scale = np.asarray(inputs["gn_scale"], dtype=np.float32)
    gn_bias = np.asarray(inputs["gn_bias"], dtype=np.float32)
    w_qkv = np.asarray(inputs["w_qkv"], dtype=np.float32)
    b_qkv = np.asarray(inputs["b_qkv"], dtype=np.float32)
    w_proj = np.asarray(inputs["w_proj"], dtype=np.float32)
    b_proj = np.asarray(inputs["b_proj"], dtype=np.float32)

    # per-head qk rows (h*192 .. h*192+128) and v rows (h*192+128 .. h*192+192)
    wqk = np.empty((C, HEADS * 128), np.float32)
    bqk = np.empty((128, HEADS), np.float32)
    wv = np.zeros((C, HEADS * 65), np.float32)
    bv = np.zeros((HEADS * 65,), np.float32)
    for h in range(HEADS):
        r0 = h * 3 * CH
        wqk[:, h * 128:(h + 1) * 128] = w_qkv[r0:r0 + 128, :].T
        bqk[:, h] = b_qkv[r0:r0 + 128]
        wv[:, h * 65:h * 65 + 64] = w_qkv[r0 + 128:r0 + 192, :].T
        bv[h * 65:h * 65 + 64] = b_qkv[r0 + 128:r0 + 192]
        bv[h * 65 + 64] = 1.0

    gmask = np.zeros((128, 8), np.float32)
    for g in range(8):
        gmask[g * 16:(g + 1) * 16, g] = 1.0

    common = {
        "wqkT": wqk.astype(ml_dtypes.bfloat16),
        "wvT": wv.astype(ml_dtypes.bfloat16),
        "wpT": np.ascontiguousarray(w_proj.T).astype(ml_dtypes.bfloat16),
        "bqk": bqk,
        "bv": bv,
        "bp": np.ascontiguousarray(b_proj.reshape(NCK, 128).T),
        "gns": np.ascontiguousarray(gn_scale.reshape(NCK, 128).T),
        "gnb": np.ascontiguousarray(gn_bias.reshape(NCK, 128).T),
        "gmask": gmask,
        "gmaskT": np.ascontiguousarray(gmask.T),
    }
    in_maps = []
    for i in range(NCORES):
        m = dict(common)
        m["x"] = np.ascontiguousarray(x[i * NB:(i + 1) * NB])
        in_maps.append(m)
    return in_maps


_NC_CACHE = None


def _get_nc():
    global _NC_CACHE
    if _NC_CACHE is None:
        _NC_CACHE = build_graph()
    return _NC_CACHE


def _run(inputs, trace=False, **kwargs):
    nc = _get_nc()
    in_maps = _pack_inputs(inputs)
    res = run_bass_kernel_spmd(nc, in_maps, core_ids=list(range(NCORES)),
                               trace=trace, **kwargs)
    outs = np.concatenate([np.asarray(res.results[i]["out"], dtype=np.float32)
                           for i in range(NCORES)], axis=0)
    return outs.reshape(B, C, 32, 32), res


def kernel(**inputs) -> np.ndarray:
    out, _ = _run(inputs, trace=False)
    return out
